# revision 4
# baseline (speedup 1.0000x reference)
"""Causal self-attention (B=4, T=2048, C=768, H=12, RoPE) on 8 TRN2 NeuronCores.

Sharding: core c -> (batch b = c//2, head-group g = c%2 of 6 heads).
Each core computes, for its batch element and its 6 heads:
    qkv^T-style projections, RoPE, causal attention, and the partial output
    projection  attn_out @ W_proj[rows of its heads].
Host sums the two partial outputs per batch and adds the (host-folded)
bias terms:  out[b] = part[2b] + part[2b+1] + b_proj + b_v @ W_proj.

v2: full bf16 operand pipeline (PSUM accumulation stays fp32).  All
matmul operands (xt, wqk, wv, wp, qt/kt, pp, vp, yt, rope tables,
swap permutation) are bf16: halves input DMA traffic and SBUF
footprint, enables the DVE 2x packed mode on SBUF-only elementwise
ops, and lifts the fp32r 4-cycles-per-row penalty on the narrow
(<256-col) diagonal-block matmuls.  rel-err budget is 2e-2; measured
bf16 error is ~4e-3.

On-chip layout (per core):
  xt   [C, T]   = x[b].T                       (bf16, matmul operand)
  wqk  [C, 768] = [Wq_g | Wk_g]                (bf16)
  wv   [C, 384] = Wv_g                         (bf16)
  wp   [384, C] = W_proj rows of group         (bf16)
  Q^T/K^T built as [128, T] "m-tiles" (2 heads each) via
  out = wqk_mtile.T @ xt  with RoPE applied by
  y = raw*CC + rowswap(raw)*SS  (rowswap via a [128,128] permutation matmul
  into the unused half of the QK PSUM tile).
  Scores are computed transposed: S^T[k, q] chunks [128, 512], exp on ACT,
  causal mask via affine_select, then Y'^T = [V|1]^T-chain accumulation
  giving both Y^T (rows 0-63) and softmax denominators (row 64).
  Diagonal 128x512 blocks are trimmed: score/attn matmuls, exp, and the
  mask only touch the causally-valid column range.
  Softmax renorm: one merged [1,1024] reciprocal per (pair, qc) covering
  both heads, DRAM-roundtrip broadcast (bf16) + renorm DMAs on the SP
  queue; input reloads stay on the GpSimd queue (drains early, hides the
  next loop iteration's reload).
"""
import sys
sys.path.insert(0, "/opt/trn_rl_repo")

import numpy as np

ROPE_BASE = 10000.0
NCORES = 8

_CACHE = {}


def _np_bf16():
    from concourse import mybir
    return mybir.dt.np(mybir.dt.bfloat16)


def _rope_tables(T):
    inv_freq = 1.0 / (ROPE_BASE ** (np.arange(0, 64, 2, dtype=np.float64) / 64))
    t = np.arange(T, dtype=np.float64)
    fr = np.outer(t, inv_freq)            # [T, 32]
    cosT = np.cos(fr).T.astype(np.float32)   # [32, T]
    sinT = np.sin(fr).T.astype(np.float32)
    CC = np.tile(cosT, (4, 1))            # [128, T]
    SS = np.concatenate([sinT, -sinT, sinT, -sinT], axis=0)  # [128, T]
    return CC, SS


def _swap_perm():
    """[128,128] permutation P with (P @ x)[p] = x[sigma(p)], sigma swapping
    halves 0-31<->32-63 and 64-95<->96-127 (the RoPE rotate-half pairing,
    applied independently to the two 64-row heads of an m-tile)."""
    P = np.zeros((128, 128), dtype=np.float32)
    for h in (0, 64):
        for p in range(32):
            # matmul computes lhsT.T @ rhs: out[m] = sum_k lhsT[k, m]*x[k]
            P[h + 32 + p, h + p] = 1.0
            P[h + p, h + 32 + p] = 1.0
    return P


def build_nc(C, T, HPC, loop_n=1):
    """Build the per-core Bass program. C: contraction dim, T: seq len,
    HPC: heads per core (even)."""
    import concourse.bass as bass
    import concourse.tile as tile
    from concourse import bacc, mybir

    F32 = mybir.dt.float32
    BF16 = mybir.dt.bfloat16
    Act = mybir.ActivationFunctionType
    npbf16 = _np_bf16()

    KT = C // 128          # contraction k-tiles
    NP = HPC // 2          # head pairs
    MT = 2 * NP            # qk m-tiles (Q tiles then K tiles)
    TT = T // 128          # 128-row t-tiles (= key chunks)
    QC = T // 512          # query chunks of 512
    VC = 64 * HPC          # v columns
    PC = VC // 128         # projection contraction k-tiles (= NP)

    nc = bacc.Bacc("TRN2", target_bir_lowering=False, debug=False)

    xt_d = nc.dram_tensor("xt", [C, T], BF16, kind="ExternalInput")
    wqk_d = nc.dram_tensor("wqk", [C, 2 * VC], BF16, kind="ExternalInput")
    bqk_d = nc.dram_tensor("bqk", [2 * VC], F32, kind="ExternalInput")
    wv_d = nc.dram_tensor("wv", [C, VC], BF16, kind="ExternalInput")
    wp_d = nc.dram_tensor("wp", [VC, C], BF16, kind="ExternalInput")
    out_d = nc.dram_tensor("out", [T, C], BF16, kind="ExternalOutput")

    rsc_d = nc.dram_tensor("rscratch", [HPC // 2, T // 512, 1024], BF16)

    CCh, SSh = _rope_tables(T)
    cc_d = nc.inline_tensor(CCh.astype(npbf16), name="rope_cc")
    ss_d = nc.inline_tensor(SSh.astype(npbf16), name="rope_ss")
    ps_d = nc.inline_tensor(_swap_perm().astype(npbf16), name="pswap")

    import contextlib

    @contextlib.contextmanager
    def _maybe_loop(tc):
        if loop_n > 1:
            with tc.For_i(0, loop_n, 1):
                yield
        else:
            yield

    with nc.allow_low_precision(reason="bf16 matmul pipeline"), \
         tile.TileContext(nc) as tc:
        with contextlib.ExitStack() as octx, _maybe_loop(tc), \
             contextlib.ExitStack() as ctx:
            # ---- long-lived pools -------------------------------------
            big = ctx.enter_context(tc.tile_pool(name="big", bufs=1))
            qk_pool = ctx.enter_context(tc.tile_pool(name="qks", bufs=2))
            vp_pool = ctx.enter_context(tc.tile_pool(name="vp", bufs=1))
            misc = ctx.enter_context(tc.tile_pool(name="misc", bufs=1))

            xt_sb = big.tile([128, KT, T], BF16, tag="bigshare")

            vp_sb = vp_pool.tile([128, TT, HPC, 65], BF16)
            bqk_sb = misc.tile([128, MT], F32)
            nc.vector.tensor_copy(
                vp_sb[:, :, :, 64:65],
                nc.const_aps.tensor(1.0, (128, TT, HPC, 1)))

            nc.sync.dma_start(
                bqk_sb[:],
                bqk_d.ap().rearrange("(m p) -> p m", p=128))

            # ---- stage B0: weights + V projection ---------------------
            QKW = 512   # QK rope chunk width
            wqk_pool = ctx.enter_context(tc.tile_pool(name="wqk", bufs=1))
            ccss_pool = ctx.enter_context(tc.tile_pool(name="ccss", bufs=1))
            raw_pool = ctx.enter_context(tc.tile_pool(name="raw", bufs=2))
            swp_pool = ctx.enter_context(tc.tile_pool(name="swp", bufs=2))

            wqk_sb = wqk_pool.tile([128, KT, 2 * VC], BF16)
            wv_pool = ctx.enter_context(tc.tile_pool(name="wv", bufs=1))
            wv_sb = wv_pool.tile([128, KT, VC], BF16)
            cc_sb = ccss_pool.tile([128, T], BF16)
            ss_sb = ccss_pool.tile([128, T], BF16)
            ps_sb = ccss_pool.tile([128, 128], BF16)
            # Input/weight reloads go on the GpSimd DMA queue: it drains
            # well before the Sync (out writes) and Scalar (last exps)
            # queues, so in looped execution the next iteration's reload
            # is already done when the first QK matmul needs it.
            # Small one-time tables (rope cc/ss, swap perm) go on Sync so
            # the first rope combine isn't gated on the full reload.
            for k in range(KT):
                nc.gpsimd.dma_start(xt_sb[:, k, :],
                                    xt_d.ap()[k * 128:(k + 1) * 128, :])
                nc.gpsimd.dma_start(wqk_sb[:, k, :],
                                    wqk_d.ap()[k * 128:(k + 1) * 128, :])
            for k in range(KT):
                nc.gpsimd.dma_start(wv_sb[:, k, :],
                                    wv_d.ap()[k * 128:(k + 1) * 128, :])
            nc.sync.dma_start(cc_sb, cc_d.ap())
            nc.sync.dma_start(ss_sb, ss_d.ap())
            nc.sync.dma_start(ps_sb, ps_d.ap())

            # ---- stage BC: per-pair QK+RoPE then attention ------------
            yt_sb = big.tile([128, NP, T], BF16, tag="yt")
            with tc.tile_pool(name="mmps", bufs=2, space="PSUM") as mmps, \
                 tc.tile_pool(name="yps", bufs=2, space="PSUM") as yps, \
                 tc.tile_pool(name="pt", bufs=3) as pt_pool, \
                 tc.tile_pool(name="ra", bufs=1) as ra_pool, \
                 tc.tile_pool(name="tb", bufs=2) as tb_pool, \
                 tc.tile_pool(name="bcs", bufs=1) as bcs_pool:
                for j in range(NP):
                    # QK + RoPE for this pair's two m-tiles
                    qt = qk_pool.tile([128, T], BF16, tag="qt", name=f"qt{j}")
                    kt = qk_pool.tile([128, T], BF16, tag="kt", name=f"kt{j}")
                    for dst, m in ((qt, j), (kt, NP + j)):
                        for ch in range(T // QKW):
                            psum = mmps.tile([128, 1024], F32, tag="mm",
                                             name="qkpsum")
                            cs2 = slice(ch * QKW, (ch + 1) * QKW)
                            for k in range(KT):
                                nc.tensor.matmul(
                                    psum[:, 0:QKW],
                                    wqk_sb[:, k, m * 128:(m + 1) * 128],
                                    xt_sb[:, k, cs2],
                                    start=(k == 0), stop=(k == KT - 1))
                            raw = raw_pool.tile([128, QKW], BF16)
                            nc.vector.tensor_scalar_add(
                                raw, psum[:, 0:QKW], bqk_sb[:, m:m + 1])
                            # rotate-half via permutation matmul into the
                            # unused half of the same PSUM tile
                            nc.tensor.matmul(psum[:, 512:512 + QKW], ps_sb,
                                             raw, start=True, stop=True)
                            swps = swp_pool.tile([128, QKW], BF16, tag="sw")
                            nc.vector.tensor_mul(swps, psum[:, 512:512 + QKW],
                                                 ss_sb[:, cs2])
                            nc.vector.tensor_mul(raw, raw, cc_sb[:, cs2])
                            nc.vector.tensor_add(dst[:, cs2], raw, swps)

                    if j == 0:
                        # V projection after pair-0 QK: overlaps attention
                        for tt in range(TT):
                            vpsum = mmps.tile([128, 1024], F32, tag="mm",
                                              name="vpsum")
                            for k in range(KT):
                                nc.tensor.matmul(
                                    vpsum[:, 0:VC],
                                    xt_sb[:, k, tt * 128:(tt + 1) * 128],
                                    wv_sb[:, k, :],
                                    start=(k == 0), stop=(k == KT - 1))
                            nc.vector.tensor_copy(
                                vp_sb[:, tt, :, 0:64],
                                vpsum[:, 0:VC].rearrange("p (h d) -> p h d",
                                                         h=HPC))

                    # attention for pair j
                    for qc in range(QC):
                        nkc = 4 * (qc + 1)
                        yab = yps.tile([65, 1024], F32, tag="yab", name="yab")
                        for kc in range(nkc):
                            diag = kc >= 4 * qc
                            lo = 128 * (kc - 4 * qc) if diag else 0
                            qs = slice(qc * 512 + lo, (qc + 1) * 512)
                            ks = slice(kc * 128, (kc + 1) * 128)
                            spair = mmps.tile([128, 1024], F32, tag="mm",
                                              name="spair")
                            nc.tensor.matmul(spair[:, lo:512], kt[0:64, ks],
                                             qt[0:64, qs], start=True, stop=True)
                            nc.tensor.matmul(spair[:, 512 + lo:1024],
                                             kt[64:128, ks],
                                             qt[64:128, qs], start=True, stop=True)
                            pp = pt_pool.tile([128, 1024], BF16, tag="pp")
                            sp3 = spair[:].rearrange("p (h q) -> p h q", h=2)
                            pp3 = pp[:].rearrange("p (h q) -> p h q", h=2)
                            nc.scalar.activation(pp3[:, :, lo:512],
                                                 sp3[:, :, lo:512],
                                                 Act.Exp, scale=0.125)
                            if diag:  # mask k > q -> 0 on the leading block
                                nc.gpsimd.affine_select(
                                    out=pp3[:, :, lo:lo + 128],
                                    in_=pp3[:, :, lo:lo + 128],
                                    compare_op=mybir.AluOpType.is_ge,
                                    fill=0.0,
                                    base=0,
                                    channel_multiplier=-1,
                                    pattern=[[0, 2], [1, 128]])
                            nc.tensor.matmul(yab[:, lo:512],
                                             vp_sb[:, kc, 2 * j, :],
                                             pp[:, lo:512],
                                             start=(kc == 0), stop=(kc == nkc - 1))
                            nc.tensor.matmul(yab[:, 512 + lo:1024],
                                             vp_sb[:, kc, 2 * j + 1, :],
                                             pp[:, 512 + lo:1024],
                                             start=(kc == 0), stop=(kc == nkc - 1))
                        qs = slice(qc * 512, (qc + 1) * 512)
                        ra = ra_pool.tile([65, 1024], BF16, tag="ra")
                        nc.vector.reciprocal(ra[64:65, :], yab[64:65, :])
                        bc = bcs_pool.tile([64, 1024], BF16, tag="bc")
                        nc.sync.dma_start(rsc_d.ap()[j, qc], ra[64:65, :])
                        nc.sync.dma_start(
                            bc,
                            rsc_d.ap()[j, qc:qc + 1, :].to_broadcast((64, 1024)))
                        nc.vector.tensor_mul(yt_sb[0:64, j, qs],
                                             yab[0:64, 0:512], bc[:, 0:512])
                        tb = tb_pool.tile([64, 512], BF16)
                        nc.vector.tensor_mul(tb, yab[0:64, 512:1024],
                                             bc[:, 512:1024])
                        nc.sync.dma_start(yt_sb[64:128, j, qs], tb)

            # ---- stage D: output projection ---------------------------
            with tc.tile_pool(name="wp", bufs=1) as wp_pool, \
                 tc.tile_pool(name="osb", bufs=3) as osb_pool, \
                 tc.tile_pool(name="pps", bufs=3, space="PSUM") as pps:
                wp_sb = wp_pool.tile([128, PC, C], BF16)
                for k in range(PC):
                    nc.sync.dma_start(wp_sb[:, k, :],
                                      wp_d.ap()[k * 128:(k + 1) * 128, :])
                ccw = 384 if C % 384 == 0 else C  # proj column chunk width
                ncc = C // ccw
                for tt in range(TT):
                    osb = osb_pool.tile([128, C], F32)
                    for cc in range(ncc):
                        cs = slice(cc * ccw, (cc + 1) * ccw)
                        psum = pps.tile([128, ccw], F32)
                        for k in range(PC):
                            nc.tensor.matmul(
                                psum, yt_sb[:, k, tt * 128:(tt + 1) * 128],
                                wp_sb[:, k, cs],
                                start=(k == 0), stop=(k == PC - 1))
                        nc.vector.tensor_copy(osb[:, cs], psum)
                    nc.sync.dma_start(out_d.ap()[tt * 128:(tt + 1) * 128, :], osb)

    nc.compile()
    return nc


class _Runner:
    """Cached-jit SPMD runner (mirrors bass2jax.run_bass_via_pjrt, reusable)."""

    def __init__(self, nc, n_cores):
        import jax
        from jax.sharding import Mesh, PartitionSpec
        from jax.experimental.shard_map import shard_map
        import concourse.mybir as mybir
        from concourse import bass2jax

        bass2jax.install_neuronx_cc_hook()
        self.n_cores = n_cores
        part_name = (nc.partition_id_tensor.name
                     if nc.partition_id_tensor is not None else None)
        in_names, out_names, out_avals, zero_outs = [], [], [], []
        for alloc in nc.m.functions[0].allocations:
            if not isinstance(alloc, mybir.MemoryLocationSet):
                continue
            name = alloc.memorylocations[0].name
            if alloc.kind == "ExternalInput":
                if name != part_name:
                    in_names.append(name)
            elif alloc.kind == "ExternalOutput":
                out_names.append(name)
                shape = tuple(alloc.tensor_shape)
                dtype = mybir.dt.np(alloc.dtype)
                out_avals.append(jax.core.ShapedArray(shape, dtype))
                zero_outs.append(np.zeros(shape, dtype))
        self.in_names, self.out_names = in_names, out_names
        self.out_avals, self.zero_outs = out_avals, zero_outs
        all_names = in_names + out_names
        if part_name is not None:
            all_names = all_names + [part_name]

        def _body(*args):
            operands = list(args)
            if part_name is not None:
                operands.append(bass2jax.partition_id_tensor())
            return tuple(bass2jax._bass_exec_p.bind(
                *operands,
                out_avals=tuple(out_avals),
                in_names=tuple(all_names),
                out_names=tuple(out_names),
                lowering_input_output_aliases=(),
                sim_require_finite=True,
                sim_require_nnan=True,
                nc=nc,
            ))

        devices = jax.devices()[:n_cores]
        mesh = Mesh(np.asarray(devices), ("core",))
        nin = len(in_names) + len(out_names)
        self._fn = jax.jit(
            shard_map(_body, mesh=mesh,
                      in_specs=(PartitionSpec("core"),) * nin,
                      out_specs=(PartitionSpec("core"),) * len(out_names),
                      check_rep=False),
            keep_unused=True)

    def run(self, in_maps):
        args = [np.concatenate([np.asarray(m[name]) for m in in_maps], axis=0)
                for name in self.in_names]
        args += [np.zeros((self.n_cores * z.shape[0], *z.shape[1:]), z.dtype)
                 for z in self.zero_outs]
        outs = self._fn(*args)
        res = []
        for c in range(self.n_cores):
            d = {}
            for i, name in enumerate(self.out_names):
                per = np.asarray(outs[i]).reshape(
                    self.n_cores, *self.out_avals[i].shape)
                d[name] = per[c]
            res.append(d)
        return res


def _run(nc, in_maps):
    key = ("runner", id(nc))
    if key not in _CACHE:
        _CACHE[key] = _Runner(nc, len(in_maps))
    import types
    return types.SimpleNamespace(results=_CACHE[key].run(in_maps))


def make_in_maps(inputs):
    npbf16 = _np_bf16()
    x = np.asarray(inputs["x"], dtype=np.float32)
    W = np.asarray(inputs["W_attn"], dtype=np.float32)
    b = np.asarray(inputs["b_attn"], dtype=np.float32)
    Wp = np.asarray(inputs["W_proj"], dtype=np.float32)

    in_maps = []
    for c in range(NCORES):
        bb, g = divmod(c, 2)
        s = 384 * g
        in_maps.append({
            "xt": np.ascontiguousarray(x[bb].T).astype(npbf16),
            "wqk": np.ascontiguousarray(
                np.concatenate([W[:, s:s + 384], W[:, 768 + s:768 + s + 384]],
                               axis=1)).astype(npbf16),
            "bqk": np.ascontiguousarray(
                np.concatenate([b[s:s + 384], b[768 + s:768 + s + 384]])),
            "wv": np.ascontiguousarray(W[:, 1536 + s:1536 + s + 384]).astype(npbf16),
            "wp": np.ascontiguousarray(Wp[s:s + 384, :]).astype(npbf16),
        })
    return in_maps


def kernel(**inputs):
    x = np.asarray(inputs["x"], dtype=np.float32)
    b = np.asarray(inputs["b_attn"], dtype=np.float32)
    Wp = np.asarray(inputs["W_proj"], dtype=np.float32)
    bp = np.asarray(inputs["b_proj"], dtype=np.float32)
    B, T, C = x.shape

    if "nc" not in _CACHE:
        _CACHE["nc"] = build_nc(C, T, 6)
    nc = _CACHE["nc"]

    in_maps = make_in_maps(inputs)

    res = _run(nc, in_maps).results
    extra = (bp + b[1536:2304] @ Wp).astype(np.float32)  # [C]
    out = np.empty((B, T, C), dtype=np.float32)
    for bb in range(B):
        out[bb] = res[2 * bb]["out"] + res[2 * bb + 1]["out"] + extra
    return out
